# revision 20
# baseline (speedup 1.0000x reference)
"""GraphSAGE 3-layer GNN forward pass on 8 Trainium2 NeuronCores.

Sharding: nodes split by range across 8 cores (graph/data parallel).
Per layer the message table z = h @ Wl is computed shard-wise (node-major
rows) and AllGathered into a replicated DRAM table; each core aggregates
the edges whose dst is in its shard: dma_gather pulls z[src] rows (256B)
into SBUF and a one-hot matmul on the tensor engine does the segment-sum
into PSUM, feature-major for all layers: the one-hot is 512 wide (a full
PSUM bank = a group of 4 dst tiles), so gather chunks only need to be
(group, bucket)-pure, not tile-pure -- edges pack densely (~4% padding
instead of ~25%).  Layer 3 accumulates [17, 512] feature-major and is
transposed per dst tile on the tensor engine before log_softmax, which
runs with per-partition biases on the scalar engine.
Mean-normalization (1/deg), the self term h @ Wr, BatchNorm (stats
AllReduced), and ReLU run on vector/scalar engines.
int16 gather indices only reach 32768 rows, so the table is processed in
4 buckets of 25088 rows with (group, bucket)-pure edge chunks.
"""

import ml_dtypes
import numpy as np

# ---------------- problem constants (hardcoded per contract) ----------------
N = 100000
E = 1600000
FIN = 200
NCORES = 8
NPC = N // NCORES            # 12500 nodes per core
NT = 98                      # dst tiles of 128 nodes per core
NPAD = NT * 128              # 12544
SHARD = NPAD                 # table rows contributed per core
TROWS = SHARD * NCORES       # 100352
NBUCK = 4
BROWS = TROWS // NBUCK       # 25088 (< 32768, int16-safe)
F1, F2, F3 = 64, 32, 17
EPS = 1e-5

# ---------------- tunables ----------------
G4 = 4                # dst tiles per PSUM bank / one-hot group
ZG = 7                # dst tiles per z-phase slab group (0-48 = 7 groups)
PBATCH = 8            # chunks per one-hot build DVE op


def _wrap16(idx_flat):
    """dma_gather index layout: position i -> partition i%16, col i//16,
    replicated across the 8 q7 core pairs (128 partitions)."""
    n = idx_flat.shape[0]
    w = idx_flat.reshape(n // 16, 16).T.copy()
    return np.tile(w, (8, 1))


def _preprocess(edge_index):
    src = np.asarray(edge_index[0], dtype=np.int64)
    dst = np.asarray(edge_index[1], dtype=np.int64)
    trow = (src // NPC) * SHARD + (src % NPC)   # global table row of src
    bucket = trow // BROWS
    rel = trow - bucket * BROWS

    dst_core = dst // NPC
    dloc = dst - dst_core * NPC
    tile_e = dloc >> 7

    groups4 = [list(range(g, min(g + G4, NT))) for g in range(0, NT, G4)]
    NG = len(groups4)

    per_core = []
    cnts_all = np.zeros((NCORES, NG, NBUCK), np.int64)
    for c in range(NCORES):
        m = dst_core == c
        g4 = tile_e[m] // G4
        key = g4 * NBUCK + bucket[m]
        order = np.argsort(key, kind="stable")
        cnts = np.bincount(key, minlength=NG * NBUCK).reshape(NG, NBUCK)
        per_core.append({
            "key": key[order],
            "rel": rel[m][order],
            "dgrel": (dloc[m] - (tile_e[m] // G4) * (G4 * 128))[order],
            "cnt": np.bincount(dloc[m], minlength=NPC),
        })
        cnts_all[c] = cnts
    csched = np.maximum((cnts_all.max(axis=0) + 127) >> 7, 1)   # [NG, NBUCK]

    chunk_start = np.zeros((NG, NBUCK), np.int64)
    calls = []  # (bucket, chunk_qstart, nchunks, group_index)
    q = 0
    for gi in range(NG):
        for b in range(NBUCK):
            chunk_start[gi, b] = q
            calls.append((b, q, int(csched[gi, b]), gi))
            q += int(csched[gi, b])
    nchunk = q

    idx_all = np.zeros((NCORES, 128, nchunk * 8), np.int16)
    dgrel_all = np.full((NCORES, 128, nchunk), -1.0, np.float32)
    rcnt_row = np.zeros((NCORES, NPAD), np.float32)
    for c in range(NCORES):
        ck = per_core[c]
        cnts = cnts_all[c]
        seg_off = np.zeros(NG * NBUCK + 1, np.int64)
        seg_off[1:] = np.cumsum(cnts.reshape(-1))
        pos = np.arange(len(ck["key"])) - seg_off[ck["key"]]
        g_e = ck["key"] // NBUCK
        b_e = ck["key"] % NBUCK
        slot = chunk_start[g_e, b_e] * 128 + pos
        idx_flat = np.zeros(nchunk * 128, np.int16)
        idx_flat[slot] = ck["rel"].astype(np.int16)
        idx_all[c] = _wrap16(idx_flat)
        dgrel_all[c][slot & 127, slot >> 7] = ck["dgrel"].astype(np.float32)
        rc_pad = np.ones(NPAD, np.float32)
        rc_pad[:NPC] = 1.0 / np.maximum(ck["cnt"], 1).astype(np.float32)
        rcnt_row[c] = rc_pad

    return {
        "groups4": groups4, "csched": csched, "calls": calls,
        "nchunk": nchunk, "chunk_start": chunk_start,
        "idx_all": idx_all, "dgrel_all": dgrel_all, "rcnt_row": rcnt_row,
    }


def _build_program(pp):
    import concourse.bacc as bacc
    import concourse.tile as tile
    import concourse.mybir as mybir

    f32 = mybir.dt.float32
    AX = mybir.AxisListType
    ALU = mybir.AluOpType
    ACT = mybir.ActivationFunctionType

    groups4 = pp["groups4"]
    calls = pp["calls"]
    csched = pp["csched"]
    nchunk = pp["nchunk"]
    NG = len(groups4)
    max_call_chunks = max(nc_ for (_, _, nc_, _) in calls)

    zgroups = [list(range(g, min(g + ZG, NT))) for g in range(0, NT, ZG)]
    ZGW = ZG * 128

    nc = bacc.Bacc("TRN2", target_bir_lowering=False, debug=False,
                   num_devices=NCORES, num_swdge_queues=4)

    # ---------------- I/O ----------------
    t_xT = nc.dram_tensor("xT", [FIN, NPAD], mybir.dt.bfloat16, kind="ExternalInput")
    t_idx = nc.dram_tensor("gidx", [128, nchunk * 8], mybir.dt.int16, kind="ExternalInput")
    t_dgrel = nc.dram_tensor("dgrel", [128, nchunk], f32, kind="ExternalInput")
    t_rcnt_fm = nc.dram_tensor("rcnt_fm", [64, NPAD], f32, kind="ExternalInput")
    t_iota512 = nc.dram_tensor("iota512", [128, 512], f32, kind="ExternalInput")
    t_ident = nc.dram_tensor("ident", [128, 128], f32, kind="ExternalInput")
    t_W1l = nc.dram_tensor("W1l", [FIN, F1], f32, kind="ExternalInput")
    t_W1r = nc.dram_tensor("W1r", [FIN, F1], f32, kind="ExternalInput")
    t_W2l = nc.dram_tensor("W2lp", [F1, 64], f32, kind="ExternalInput")
    t_W2r = nc.dram_tensor("W2r", [F1, F2], f32, kind="ExternalInput")
    t_W3l = nc.dram_tensor("W3lp", [F2, 64], f32, kind="ExternalInput")
    t_W3r = nc.dram_tensor("W3r", [F2, F3], f32, kind="ExternalInput")
    t_g1 = nc.dram_tensor("g1", [F1, 1], f32, kind="ExternalInput")
    t_be1 = nc.dram_tensor("be1", [F1, 1], f32, kind="ExternalInput")
    t_g2 = nc.dram_tensor("g2", [F2, 1], f32, kind="ExternalInput")
    t_be2 = nc.dram_tensor("be2", [F2, 1], f32, kind="ExternalInput")
    t_b3 = nc.dram_tensor("b3col", [F3, 1], f32, kind="ExternalInput")
    t_out = nc.dram_tensor("out", [NPAD, F3], f32, kind="ExternalOutput")

    bf16 = mybir.dt.bfloat16
    shard1 = nc.dram_tensor("shard1", [SHARD, 128], bf16, kind="Internal")
    shard2 = nc.dram_tensor("shard2", [SHARD, 128], bf16, kind="Internal")
    shard3 = nc.dram_tensor("shard3", [SHARD, 128], bf16, kind="Internal")
    zfull1 = nc.dram_tensor("zfull1", [TROWS, 128], bf16, kind="Internal", addr_space="Shared")
    zfull2 = nc.dram_tensor("zfull2", [TROWS, 128], bf16, kind="Internal", addr_space="Shared")
    zfull3 = nc.dram_tensor("zfull3", [TROWS, 128], bf16, kind="Internal", addr_space="Shared")
    zrT1_d = nc.dram_tensor("zrT1", [64, NPAD], f32, kind="Internal")
    zrT2_d = nc.dram_tensor("zrT2", [F2, NPAD], f32, kind="Internal")
    zrT3_d = nc.dram_tensor("zrT3", [F3, NPAD], f32, kind="Internal")
    hT1_d = nc.dram_tensor("hT1", [64, NPAD], f32, kind="Internal")
    hT2_d = nc.dram_tensor("hT2", [F2, NPAD], f32, kind="Internal")
    bn_in1 = nc.dram_tensor("bn_in1", [F1, 2], f32, kind="Internal")
    bn_out1 = nc.dram_tensor("bn_out1", [F1, 2], f32, kind="Internal", addr_space="Shared")
    bn_in2 = nc.dram_tensor("bn_in2", [F2, 2], f32, kind="Internal")
    bn_out2 = nc.dram_tensor("bn_out2", [F2, 2], f32, kind="Internal", addr_space="Shared")

    RG = [list(range(NCORES))]

    with tile.TileContext(nc) as tc:
        with tc.tile_pool(name="const", bufs=1) as constp, \
             tc.tile_pool(name="wpool", bufs=1) as wpool, \
             tc.tile_pool(name="stage", bufs=3) as stagep, \
             tc.tile_pool(name="sm3", bufs=6) as sm3p, \
             tc.tile_pool(name="slab", bufs=3) as slabp, \
             tc.tile_pool(name="gbuf", bufs=6) as gbufp, \
             tc.tile_pool(name="pbuf", bufs=3) as pbufp, \
             tc.tile_pool(name="zpsum", bufs=3, space="PSUM") as zpsum, \
             tc.tile_pool(name="spsum", bufs=5, space="PSUM") as spsum, \
             tc.tile_pool(name="small", bufs=1) as smallp:

            # ---- constants
            iota512 = constp.tile([128, 512], f32)
            nc.sync.dma_start(iota512[:], t_iota512.ap())
            ident = constp.tile([128, 128], f32)
            nc.sync.dma_start(ident[:], t_ident.ap())
            idx_t = constp.tile([128, nchunk * 8], mybir.dt.int16)
            nc.sync.dma_start(idx_t[:], t_idx.ap())
            dgrel_t = constp.tile([128, nchunk], f32)
            nc.sync.dma_start(dgrel_t[:], t_dgrel.ap())

            def wload(name, tt, shape, dt=f32):
                w = wpool.tile(shape, dt, tag=name)
                if dt is f32:
                    nc.sync.dma_start(w[:], tt)
                else:
                    nc.gpsimd.dma_start(out=w[:], in_=tt)
                return w

            W1l_a = wload("w1la", t_W1l.ap()[:128], [128, F1], bf16)
            W1l_b = wload("w1lb", t_W1l.ap()[128:], [72, F1], bf16)
            W1r_a = wload("w1ra", t_W1r.ap()[:128], [128, F1], bf16)
            W1r_b = wload("w1rb", t_W1r.ap()[128:], [72, F1], bf16)
            W2l_t = wload("w2l", t_W2l.ap(), [F1, 64], bf16)
            W2r_t = wload("w2r", t_W2r.ap(), [F1, F2], bf16)
            W3l_t = wload("w3l", t_W3l.ap(), [F2, 64], bf16)
            W3r_t = wload("w3r", t_W3r.ap(), [F2, F3], bf16)
            g1_t = wload("g1", t_g1.ap(), [F1, 1])
            be1_t = wload("be1", t_be1.ap(), [F1, 1])
            g2_t = wload("g2", t_g2.ap(), [F2, 1])
            be2_t = wload("be2", t_be2.ap(), [F2, 1])
            b3_t = wload("b3", t_b3.ap(), [F3, 1])

            # ================= layer-1 z phase =================
            with nc.named_scope("L1z"):
                for g in zgroups:
                    gw = len(g) * 128
                    c0 = g[0] * 128
                    xa = slabp.tile([128, ZGW], bf16, tag="xa")
                    xb = slabp.tile([72, ZGW], bf16, tag="xb")
                    nc.sync.dma_start(xa[:, :gw], t_xT.ap()[:128, c0:c0 + gw])
                    nc.sync.dma_start(xb[:, :gw], t_xT.ap()[128:, c0:c0 + gw])
                    zr_sl = stagep.tile([64, ZGW], f32, tag="zrslab")
                    for ti, t in enumerate(g):
                        xs_a = xa[:, ti * 128:(ti + 1) * 128]
                        xs_b = xb[:, ti * 128:(ti + 1) * 128]
                        pz = zpsum.tile([128, 128], f32, tag="zps")
                        nc.tensor.matmul(pz[:, :F1], xs_a, W1l_a[:], start=True, stop=False)
                        nc.tensor.matmul(pz[:, :F1], xs_b, W1l_b[:], start=False, stop=True)
                        zs = sm3p.tile([128, 128], bf16, tag="zstage")
                        nc.scalar.copy(zs[:, 0:64], pz[:, :F1])
                        nc.vector.tensor_copy(zs[:, 64:128], pz[:, :F1])
                        nc.sync.dma_start(shard1.ap()[t * 128:(t + 1) * 128], zs[:])
                        pr = zpsum.tile([128, 128], f32, tag="zps")
                        nc.tensor.matmul(pr[:F1, :], W1r_a[:], xs_a, start=True, stop=False)
                        nc.tensor.matmul(pr[:F1, :], W1r_b[:], xs_b, start=False, stop=True)
                        nc.vector.tensor_copy(zr_sl[:, ti * 128:(ti + 1) * 128], pr[:F1, :])
                    nc.sync.dma_start(zrT1_d.ap()[:, c0:c0 + gw], zr_sl[:, :gw])

            with nc.named_scope("AG1"):
                nc.gpsimd.collective_compute(
                    "AllGather", ALU.bypass, replica_groups=RG,
                    ins=[shard1.ap()], outs=[zfull1.ap()])

            # ========== generic gather/aggregate (feature-major) ==========
            def agg_layer(zfull, Fw, zr_src, h_sink, scope, final_cb=None):
                """final_cb(ps, gi, g) consumes the [Fw, W] psum accumulator;
                when None the standard L1/L2 finale (rcnt, self term, BN
                stats, h_sink) runs."""
                stat_parts = []
                K = 4
                call_by = {(gi_, b_): (b_, qs_, nch_, gi_)
                           for (b_, qs_, nch_, gi_) in calls}
                issue = []
                for gi_ in range(NG):
                    issue.append(call_by[(gi_, 0)])
                    issue.append(call_by[(gi_, 1)])
                    if gi_ >= K:
                        issue.append(call_by[(gi_ - K, 2)])
                        issue.append(call_by[(gi_ - K, 3)])
                for gi_ in range(NG - K, NG):
                    issue.append(call_by[(gi_, 2)])
                    issue.append(call_by[(gi_, 3)])
                with nc.named_scope(scope):
                    ps_by_g = {}
                    for ci_, (b, qs, nch, gi) in enumerate(issue):
                        g = groups4[gi]
                        W = len(g) * 128
                        c0 = g[0] * 128
                        if b == 0:
                            ps_by_g[gi] = spsum.tile([64, 512], f32, tag="sacc", name=f"sacc_{scope}_{gi}")
                        ps = ps_by_g[gi]
                        gb = gbufp.tile([128, max_call_chunks, 128], bf16, tag="gb")
                        nc.gpsimd.dma_gather(
                            out_ap=gb[:, :nch, :],
                            in_ap=zfull.ap()[b * BROWS:(b + 1) * BROWS],
                            idxs_ap=idx_t[:, qs * 8:(qs + nch) * 8],
                            num_idxs=nch * 128, num_idxs_reg=nch * 128,
                            elem_size=128, single_packet=False,
                            queue_num=ci_ % 4)
                        qoff = 0
                        while qoff < nch:
                            gsz = min(PBATCH, nch - qoff)
                            P = pbufp.tile([128, PBATCH * 512], bf16, tag="P")
                            nc.vector.tensor_tensor(
                                out=P[:, :gsz * 512].rearrange("p (g v) -> p g v", g=gsz),
                                in0=dgrel_t[:, qs + qoff:qs + qoff + gsz]
                                    .to_broadcast([128, gsz, 512]),
                                in1=iota512[:].rearrange("p (o v) -> p o v", o=1)
                                    .to_broadcast([128, gsz, 512]),
                                op=ALU.is_equal)
                            for j in range(gsz):
                                qq = qs + qoff + j
                                first = (b == 0) and (qq == qs)
                                last = (b == NBUCK - 1) and \
                                    (qq == qs + csched[gi, NBUCK - 1] - 1)
                                nc.tensor.matmul(
                                    ps[:Fw, :W],
                                    gb[:, qoff + j, 0:Fw],
                                    P[:, j * 512:j * 512 + W],
                                    start=first, stop=last, skip_group_check=True)
                            qoff += gsz
                        if b == NBUCK - 1:
                            del ps_by_g[gi]
                            if final_cb is not None:
                                final_cb(ps, gi, g)
                                continue
                            gw = W
                            rc_sl = slabp.tile([64, 512], f32, tag="rcsl")
                            nc.sync.dma_start(rc_sl[:Fw, :gw], t_rcnt_fm.ap()[:Fw, c0:c0 + gw])
                            zr_sl2 = slabp.tile([64, 512], f32, tag="zrsl2")
                            nc.sync.dma_start(zr_sl2[:Fw, :gw], zr_src[:, c0:c0 + gw])
                            hsl = stagep.tile([64, 512], f32, tag="hsl")
                            nc.vector.tensor_mul(hsl[:Fw, :gw], ps[:Fw, :gw], rc_sl[:Fw, :gw])
                            nc.vector.tensor_add(hsl[:Fw, :gw], hsl[:Fw, :gw], zr_sl2[:Fw, :gw])
                            s_p = smallp.tile([Fw, 2], f32, tag=f"stat_{scope}_{gi}")
                            nc.vector.tensor_reduce(s_p[:, 0:1], hsl[:Fw, :gw],
                                                    axis=AX.X, op=ALU.add)
                            sq_scr = stagep.tile([64, 512], f32, tag="sqscr")
                            nc.scalar.activation(sq_scr[:Fw, :gw], hsl[:Fw, :gw],
                                                 ACT.Square, accum_out=s_p[:, 1:2])
                            stat_parts.append(s_p)
                            nc.sync.dma_start(h_sink.ap()[:, c0:c0 + gw], hsl[:Fw, :gw])
                return stat_parts

            def bn_finalize(stat_parts, Fw, bn_in, bn_out, g_t, be_t, scope):
                with nc.named_scope(scope):
                    np_ = len(stat_parts)
                    stk = smallp.tile([Fw, 2 * np_], f32, tag=f"stk_{scope}")
                    for i, s_p in enumerate(stat_parts):
                        nc.vector.tensor_copy(stk[:, 2 * i:2 * i + 2], s_p[:])
                    tot = smallp.tile([Fw, 2], f32, tag=f"tot_{scope}")
                    v = stk[:].rearrange("f (i two) -> f two i", two=2)
                    nc.vector.tensor_reduce(tot[:, 0:1], v[:, 0:1, :], axis=AX.X, op=ALU.add)
                    nc.vector.tensor_reduce(tot[:, 1:2], v[:, 1:2, :], axis=AX.X, op=ALU.add)
                    nc.sync.dma_start(bn_in.ap(), tot[:])
                    nc.gpsimd.collective_compute(
                        "AllReduce", ALU.add, replica_groups=RG,
                        ins=[bn_in.ap()], outs=[bn_out.ap()])
                    red = smallp.tile([Fw, 2], f32, tag=f"red_{scope}")
                    nc.sync.dma_start(red[:], bn_out.ap())
                    mean = smallp.tile([Fw, 1], f32, tag=f"mean_{scope}")
                    nc.vector.tensor_scalar_mul(mean[:], red[:, 0:1], 1.0 / N)
                    ex2 = smallp.tile([Fw, 1], f32, tag=f"ex2_{scope}")
                    nc.vector.tensor_scalar_mul(ex2[:], red[:, 1:2], 1.0 / N)
                    var = smallp.tile([Fw, 1], f32, tag=f"var_{scope}")
                    nc.vector.tensor_mul(var[:], mean[:], mean[:])
                    nc.vector.tensor_sub(var[:], ex2[:], var[:])
                    nc.vector.tensor_scalar_add(var[:], var[:], EPS)
                    std = smallp.tile([Fw, 1], f32, tag=f"std_{scope}")
                    nc.scalar.sqrt(std[:], var[:])
                    rstd = smallp.tile([Fw, 1], f32, tag=f"rstd_{scope}")
                    nc.vector.reciprocal(rstd[:], std[:])
                    scal = smallp.tile([Fw, 1], f32, tag=f"scal_{scope}")
                    nc.vector.tensor_mul(scal[:], g_t[:], rstd[:])
                    shift = smallp.tile([Fw, 1], f32, tag=f"shift_{scope}")
                    nc.vector.tensor_mul(shift[:], mean[:], scal[:])
                    nc.vector.tensor_sub(shift[:], be_t[:], shift[:])
                    return scal, shift

            stats1 = agg_layer(zfull1, F1, zrT1_d.ap(), hT1_d, "L1agg")
            scal1, shift1 = bn_finalize(stats1, F1, bn_in1, bn_out1, g1_t, be1_t, "BN1")

            # ================= layer-2 z phase =================
            with nc.named_scope("L2z"):
                for g in zgroups:
                    gw = len(g) * 128
                    c0 = g[0] * 128
                    hs = slabp.tile([64, ZGW], f32, tag="hs")
                    nc.sync.dma_start(hs[:F1, :gw], hT1_d.ap()[:, c0:c0 + gw])
                    hsb = slabp.tile([64, ZGW], bf16, tag="hsb")
                    nc.scalar.activation(hsb[:F1, :gw], hs[:F1, :gw], ACT.Relu,
                                         bias=shift1[:], scale=scal1[:])
                    if g[-1] == NT - 1:
                        nc.vector.memzero(hsb[:F1, NPC - c0:gw])
                    zr_sl = stagep.tile([64, ZGW], f32, tag="zrslab")
                    for ti, t in enumerate(g):
                        hst = hsb[:F1, ti * 128:(ti + 1) * 128]
                        pz = zpsum.tile([128, 128], f32, tag="zps")
                        nc.tensor.matmul(pz[:, :64], hst, W2l_t[:], start=True, stop=True)
                        zs = sm3p.tile([128, 128], bf16, tag="zstage")
                        nc.scalar.copy(zs[:, 0:64], pz[:, :64])
                        nc.vector.tensor_copy(zs[:, 64:128], pz[:, :64])
                        nc.sync.dma_start(shard2.ap()[t * 128:(t + 1) * 128], zs[:])
                        pr = zpsum.tile([128, 128], f32, tag="zps")
                        nc.tensor.matmul(pr[:F2, :], W2r_t[:], hst, start=True, stop=True)
                        nc.vector.tensor_copy(zr_sl[:F2, ti * 128:(ti + 1) * 128], pr[:F2, :])
                    nc.sync.dma_start(zrT2_d.ap()[:, c0:c0 + gw], zr_sl[:F2, :gw])

            with nc.named_scope("AG2"):
                nc.gpsimd.collective_compute(
                    "AllGather", ALU.bypass, replica_groups=RG,
                    ins=[shard2.ap()], outs=[zfull2.ap()])

            stats2 = agg_layer(zfull2, F2, zrT2_d.ap(), hT2_d, "L2agg")
            scal2, shift2 = bn_finalize(stats2, F2, bn_in2, bn_out2, g2_t, be2_t, "BN2")

            # ================= layer-3 z phase =================
            with nc.named_scope("L3z"):
                for g in zgroups:
                    gw = len(g) * 128
                    c0 = g[0] * 128
                    hs = slabp.tile([64, ZGW], f32, tag="hs")
                    nc.sync.dma_start(hs[:F2, :gw], hT2_d.ap()[:, c0:c0 + gw])
                    hsb = slabp.tile([64, ZGW], bf16, tag="hsb")
                    nc.scalar.activation(hsb[:F2, :gw], hs[:F2, :gw], ACT.Relu,
                                         bias=shift2[:], scale=scal2[:])
                    if g[-1] == NT - 1:
                        nc.vector.memzero(hsb[:F2, NPC - c0:gw])
                    zr_sl = stagep.tile([64, ZGW], f32, tag="zrslab")
                    for ti, t in enumerate(g):
                        hst = hsb[:F2, ti * 128:(ti + 1) * 128]
                        pz = zpsum.tile([128, 128], f32, tag="zps")
                        nc.tensor.matmul(pz[:, :64], hst, W3l_t[:], start=True, stop=True)
                        zs = sm3p.tile([128, 128], bf16, tag="zstage")
                        nc.scalar.copy(zs[:, 0:64], pz[:, :64])
                        nc.vector.tensor_copy(zs[:, 64:128], pz[:, :64])
                        nc.sync.dma_start(shard3.ap()[t * 128:(t + 1) * 128], zs[:])
                        pr = zpsum.tile([128, 128], f32, tag="zps")
                        nc.tensor.matmul(pr[:F3, :], W3r_t[:], hst, start=True, stop=True)
                        nc.vector.tensor_copy(zr_sl[:F3, ti * 128:(ti + 1) * 128], pr[:F3, :])
                    nc.sync.dma_start(zrT3_d.ap()[:, c0:c0 + gw], zr_sl[:F3, :gw])

            with nc.named_scope("AG3"):
                nc.gpsimd.collective_compute(
                    "AllGather", ALU.bypass, replica_groups=RG,
                    ins=[shard3.ap()], outs=[zfull3.ap()])

            def l3_final(ps, gi, g):
                W = len(g) * 128
                c0 = g[0] * 128
                rc_sl = slabp.tile([64, 512], f32, tag="rcsl")
                nc.sync.dma_start(rc_sl[:F3, :W], t_rcnt_fm.ap()[:F3, c0:c0 + W])
                zr_sl2 = slabp.tile([64, 512], f32, tag="zrsl2")
                nc.sync.dma_start(zr_sl2[:F3, :W], zrT3_d.ap()[:, c0:c0 + W])
                h3f = stagep.tile([32, 512], f32, tag="h3f")
                nc.vector.tensor_mul(h3f[:F3, :W], ps[:F3, :W], rc_sl[:F3, :W])
                nc.vector.tensor_add(h3f[:F3, :W], h3f[:F3, :W], zr_sl2[:F3, :W])
                nc.scalar.activation(h3f[:F3, :W], h3f[:F3, :W], ACT.Identity,
                                     bias=b3_t[:])
                for ti, t in enumerate(g):
                    tp = zpsum.tile([128, 128], f32, tag="zps")
                    nc.tensor.transpose(tp[:, :F3],
                                        h3f[:F3, ti * 128:(ti + 1) * 128],
                                        ident[:F3, :F3])
                    h3 = sm3p.tile([128, F3], f32, tag="h3")
                    nc.scalar.copy(h3[:], tp[:, :F3])
                    mx = sm3p.tile([128, 1], f32, tag="mx")
                    nc.vector.tensor_reduce(mx[:], h3[:], axis=AX.X, op=ALU.max)
                    nmx = sm3p.tile([128, 1], f32, tag="nmx")
                    nc.vector.tensor_scalar_mul(nmx[:], mx[:], -1.0)
                    ex = sm3p.tile([128, F3], f32, tag="ex")
                    se = sm3p.tile([128, 1], f32, tag="se")
                    nc.scalar.activation(ex[:], h3[:], ACT.Exp,
                                         bias=nmx[:], accum_out=se[:])
                    ls = sm3p.tile([128, 1], f32, tag="ls")
                    nc.scalar.activation(ls[:], se[:], ACT.Ln)
                    shf = sm3p.tile([128, 1], f32, tag="shf")
                    nc.vector.tensor_sub(shf[:], nmx[:], ls[:])
                    ho = sm3p.tile([128, F3], f32, tag="ho")
                    nc.scalar.activation(ho[:], h3[:], ACT.Identity, bias=shf[:])
                    nc.sync.dma_start(t_out.ap()[t * 128:(t + 1) * 128], ho[:])

            agg_layer(zfull3, F3, None, None, "L3agg", final_cb=l3_final)

    nc.compile()
    return nc


_PROG_CACHE = {}


def _in_maps(pp, inputs):
    x = np.asarray(inputs["x"], np.float32)
    iota512 = np.broadcast_to(np.arange(512, dtype=np.float32)[None, :], (128, 512)).copy()
    ident = np.eye(128, dtype=np.float32)
    W2lp = np.zeros((F1, 64), np.float32)
    W2lp[:, :F2] = np.asarray(inputs["W2l"], np.float32)
    W3lp = np.zeros((F2, 64), np.float32)
    W3lp[:, :F3] = np.asarray(inputs["W3l"], np.float32)
    common = {
        "iota512": iota512,
        "ident": ident,
        "W1l": np.asarray(inputs["W1l"], np.float32),
        "W1r": np.asarray(inputs["W1r"], np.float32),
        "W2lp": W2lp,
        "W2r": np.asarray(inputs["W2r"], np.float32),
        "W3lp": W3lp,
        "W3r": np.asarray(inputs["W3r"], np.float32),
        "g1": np.asarray(inputs["g1"], np.float32)[:, None].copy(),
        "be1": np.asarray(inputs["be1"], np.float32)[:, None].copy(),
        "g2": np.asarray(inputs["g2"], np.float32)[:, None].copy(),
        "be2": np.asarray(inputs["be2"], np.float32)[:, None].copy(),
        "b3col": np.asarray(inputs["b3"], np.float32)[:, None].copy(),
    }
    in_maps = []
    for c in range(NCORES):
        xT = np.zeros((FIN, NPAD), ml_dtypes.bfloat16)
        xT[:, :NPC] = x[c * NPC:(c + 1) * NPC].T.astype(ml_dtypes.bfloat16)
        m = dict(common)
        m["xT"] = xT
        m["gidx"] = pp["idx_all"][c]
        m["dgrel"] = pp["dgrel_all"][c]
        m["rcnt_fm"] = np.broadcast_to(pp["rcnt_row"][c][None, :], (64, NPAD)).copy()
        in_maps.append(m)
    return in_maps


def kernel(**inputs):
    edge_index = np.asarray(inputs["edge_index"])
    pp = _preprocess(edge_index)
    key = (pp["nchunk"], pp["csched"].tobytes())
    if key not in _PROG_CACHE:
        _PROG_CACHE[key] = _build_program(pp)
    nc = _PROG_CACHE[key]
    in_maps = _in_maps(pp, inputs)
    from concourse.bass_utils import run_bass_kernel_spmd
    res = run_bass_kernel_spmd(nc, in_maps, core_ids=list(range(NCORES)))
    return np.concatenate([res.results[c]["out"][:NPC] for c in range(NCORES)], axis=0)
